# revision 37
# baseline (speedup 1.0000x reference)
"""Trainium2 Bass kernel for AudioPreprocessingLayer.

Computes: floor(log2(mel_fb @ (rfft(x*hamming, norm=forward).real ** 2)))
for x of shape (4096, 32, 512), sharded batch-wise across 8 NeuronCores.

Key ideas:
  - rfft(.).real is a matmul with the cosine matrix C[n,k] = cos(2*pi*k*n/512)/512.
  - Parity fold: C[n+256, k] = (-1)^k C[n, k], so the even-k bins need only
    ue[n] = hw[n]x[n] + hw[n+256]x[n+256] and the odd-k bins only
    uo[n] = hw[n]x[n] - hw[n+256]x[n+256] — a 256-long contraction instead
    of 512: the DFT matmul work halves.
  - Window-in-weights: ue = hw_lo * (x_lo + g*x_hi) with g = hw_hi/hw_lo,
    and the outer hw_lo folds into the cosine weights. The even side is
    folded on the DVE (one 4x-mode multiply + one 2x-mode add per chunk);
    the odd side runs UNFOLDED on the PE straight from the input tiles —
    a full fold would make the DVE the bottleneck.
  - The host hands the kernel x already TRANSPOSED to [n, r] layout (a pure
    permutation, done during sharding), so no on-chip transpose is needed:
    the DMA-loaded tiles feed the DFT matmul directly with n on partitions.
  - The row order within each DMA macro-block is permuted host-side so the
    OUTPUT rows land partition-contiguous (big store descriptors).
  - fp16 end-to-end for x/u and the windowed cosine weights (better
    precision than a bf16 pipeline and full PE speed); mag/filterbank in
    bf16 (fp16 would flush y^2 subnormals); PSUM accumulation in f32.
  - floor(log2(m)): the filterbank is pre-scaled by 2^-75, so f32 subnormal
    flush implements the eps clamp and the result is just
    (bitcast_int32(mels) >> 23) - 52, stored as bf16 (exact small ints).
"""

import os
import sys

for _p in ("/opt/trn_rl_repo",):
    if _p not in sys.path and os.path.isdir(_p):
        sys.path.append(_p)

import numpy as np
import ml_dtypes

import concourse.bass as bass
from concourse import bacc, mybir
from concourse.tile import TileContext
from concourse.bass_utils import run_bass_kernel_spmd

N_CORES = 8
B, T, FRAME = 4096, 32, 512
R = (B // N_CORES) * T  # 16384 rows of length 512 per core
N_MELS = 20
NQ = FRAME // 128  # 4 n-chunks of the transposed input
GR = 512  # rows per compute group (one PSUM bank of f32)
CHUNK_ORDER = [0, 2, 1, 3]  # n-chunk storage order: fold pairs adjacent

# DMA macro-blocks (rows): small first blocks so the pipeline fills quickly,
# and a smaller last block so the drain tail is short.
MACROS = [(0, 256), (256, 768), (1024, 1024)] + [
    (2048 + 2048 * i, 2048) for i in range(6)
] + [(14336, 1024), (15360, 1024)]
assert sum(rb for _, rb in MACROS) == R

f32 = mybir.dt.float32
f16 = mybir.dt.float16
bf16 = mybir.dt.bfloat16
i32 = mybir.dt.int32


def build_graph():
    """SPMD Bass graph for one core's shard.

    xt:  [NQ, 128, R] f16   transposed rows, n-chunks stored in order
         [0,2,1,3] so each half [0,2] / [1,3] is one contiguous DMA that
         feeds one fold: xt[i, p, r] = x[perm(r), 128*chunk(i)+p]
    ce:  [2, 128, 128] f16  diag(hw_lo) @ cos matrix, even k (2,4,...,256)
    wo:  [NQ, 128, 128] f16 full windowed cos matrix, odd k (1,3,...,255),
         n-chunks in the same [0,2,1,3] order as xt
    fbt: [2, 128, N_MELS] bf16  mel filterbank * 2^-75, split by k parity
         (the 2^-75 bias makes f32 subnormal flush implement the eps clamp:
          floor(log2(mels)) = (expbits(mels * 2^-75) >> 23) - 52, exact for
          mels > 2^-51, and the mels==0 -> eps path lands on -52 via the
          zero/subnormal exponent field)
    gr:  [128, 2] f32       window ratio hw_hi/hw_lo, n-chunked
    out: [R, N_MELS] bf16   (exact small ints; host converts to f32)
    """
    nc = bacc.Bacc(None, target_bir_lowering=False)
    xt_d = nc.declare_dram_parameter("xt", [NQ, 128, R], f16, isOutput=False)
    ce_d = nc.declare_dram_parameter("ce", [2, 128, 128], f16, isOutput=False)
    wo_d = nc.declare_dram_parameter("wo", [NQ, 128, 128], f16, isOutput=False)
    fbt_d = nc.declare_dram_parameter("fbt", [2, 128, N_MELS], bf16, isOutput=False)
    g_d = nc.declare_dram_parameter("gr", [128, 2], f32, isOutput=False)
    out_d = nc.declare_dram_parameter("out", [R, N_MELS], bf16, isOutput=True)

    with TileContext(nc) as tc:
        with (
            tc.tile_pool(name="consts", bufs=1) as consts,
            tc.tile_pool(name="xta", bufs=4) as xta_pool,
            tc.tile_pool(name="xtb", bufs=4) as xtb_pool,
            tc.tile_pool(name="gx", bufs=3) as gx_pool,
            tc.tile_pool(name="u", bufs=3) as u_pool,
            tc.tile_pool(name="mag", bufs=3) as mag_pool,
            tc.tile_pool(name="fin", bufs=2) as fin_pool,
            tc.tile_pool(name="ps_y", bufs=3, space="PSUM") as ps_y_pool,
            tc.tile_pool(name="ps_m", bufs=2, space="PSUM") as ps_m_pool,
        ):
            # gr first (unblocks the folds), then matmul weights; macro
            # input DMAs follow right behind these on the sync queue.
            g_sb = consts.tile([128, 2], f32)
            nc.sync.dma_start(out=g_sb, in_=g_d[:, :])
            wo_sb = consts.tile([128, NQ, 128], f16)
            nc.sync.dma_start(out=wo_sb, in_=wo_d.rearrange("c p k -> p c k"))
            ce_sb = consts.tile([128, 2, 128], f16)
            nc.sync.dma_start(out=ce_sb, in_=ce_d.rearrange("c p k -> p c k"))
            fbt_sb = consts.tile([128, 2, N_MELS], bf16)
            nc.sync.dma_start(out=fbt_sb, in_=fbt_d.rearrange("e j m -> j e m"))

            # three DMA dispatch queues round-robin: a queue's descriptor
            # generation (~1us) serializes with its own transfer, so two
            # streams alone leave HBM idle ~25% of the time.
            dma_q = [nc.gpsimd, nc.scalar, nc.sync]

            def emit_load(m):
                """Two half-DMAs + even-side window/fold for macro m. Each
                half carries the (x_lo, x_hi) pair one fold needs, so the
                folds start after half the macro's data has landed."""
                r0, RB = MACROS[m]
                xta_sb = xta_pool.tile([128, 2, RB], f16, name="xta_sb")
                dma_q[(2 * m) % 3].dma_start(
                    out=xta_sb,
                    in_=xt_d[0:2, :, r0 : r0 + RB].rearrange("c p r -> p c r"),
                )
                xtb_sb = xtb_pool.tile([128, 2, RB], f16, name="xtb_sb")
                dma_q[(2 * m + 1) % 3].dma_start(
                    out=xtb_sb,
                    in_=xt_d[2:4, :, r0 : r0 + RB].rearrange("c p r -> p c r"),
                )
                # u[c] = x[c] + g[c]*x[c+2]  (hw_lo is folded into ce;
                # the odd side runs unfolded on the PE straight from xt)
                gx_sb = gx_pool.tile([128, 2, RB], f16, name="gx_sb")
                u_sb = u_pool.tile([128, 2, RB], f16, name="u_sb")
                for c, h_sb in ((0, xta_sb), (1, xtb_sb)):
                    nc.vector.tensor_scalar(
                        gx_sb[:, c], h_sb[:, 1], g_sb[:, c : c + 1],
                        None, mybir.AluOpType.mult,
                    )
                    nc.vector.tensor_add(u_sb[:, c], h_sb[:, 0], gx_sb[:, c])
                return (xta_sb, xtb_sb), u_sb

            def emit_groups(m, xt_sb, u_sb):
                xta_sb, xtb_sb = xt_sb
                r0, RB = MACROS[m]
                S = RB // 128  # output slots per macro
                mels_ps = ps_m_pool.tile([128, S * N_MELS], f32, name="mels_ps")
                for off in range(0, RB, GR):
                    gr_n = min(GR, RB - off)
                    r = slice(off, off + gr_n)
                    # DFT: y[k, r] for even/odd k (f32 PSUM accumulate);
                    # odd first — it reads xt directly, no DVE dependency.
                    y_ps = ps_y_pool.tile([128, 2, gr_n], f32, name="y_ps")
                    odd_srcs = (xta_sb[:, 0, r], xta_sb[:, 1, r],
                                xtb_sb[:, 0, r], xtb_sb[:, 1, r])
                    for c in range(NQ):
                        nc.tensor.matmul(
                            y_ps[:, 1, :], wo_sb[:, c, :], odd_srcs[c],
                            start=(c == 0), stop=(c == NQ - 1),
                        )
                    for c in range(2):
                        nc.tensor.matmul(
                            y_ps[:, 0, :], ce_sb[:, c, :], u_sb[:, c, r],
                            start=(c == 0), stop=(c == 1),
                        )
                    # mag = y^2 (fused PSUM -> SBUF bf16)
                    mag_sb = mag_pool.tile([128, 2, gr_n], bf16, name="mag_sb")
                    nc.scalar.activation(
                        mag_sb, y_ps, mybir.ActivationFunctionType.Square
                    )
                    # mel: mels[r, m] += mag[k, r].T @ fbt[k, m]
                    # (a whole macro's mels fit one PSUM bank)
                    for j in range(gr_n // 128):
                        jj = slice(j * 128, (j + 1) * 128)
                        s = off // 128 + j
                        for e in range(2):
                            nc.tensor.matmul(
                                mels_ps[:, s * N_MELS : (s + 1) * N_MELS],
                                mag_sb[:, e, jj], fbt_sb[:, e, :],
                                start=(e == 0), stop=(e == 1),
                            )
                # finalize: floor(log2(mels)) = expbits(mels * 2^-75) - 52
                e_sb = fin_pool.tile([128, S * N_MELS], i32, tag="e_sb",
                                     name="e_sb")
                nc.vector.tensor_scalar(
                    e_sb,
                    mels_ps.bitcast(i32),
                    23,
                    None,
                    mybir.AluOpType.logical_shift_right,
                )
                o_sb = fin_pool.tile([128, S * N_MELS], bf16, tag="o_sb",
                                     name="o_sb")
                nc.vector.tensor_scalar_sub(o_sb, e_sb, 52.0)
                # store: rows r0 + p*S + s are partition-contiguous in DRAM
                q = dma_q[m % 3]
                q.dma_start(
                    out=out_d[r0 : r0 + RB, :].rearrange(
                        "(p j) m -> p (j m)", j=S
                    ),
                    in_=o_sb,
                )

            # software pipeline: load macro m+1 (DMA + DVE folds) before
            # emitting macro m's matmul groups, so the DVE FIFO never parks
            # next macro's folds behind this macro's exponent shifts.
            pending = {0: emit_load(0)}
            for m in range(len(MACROS)):
                if m + 1 < len(MACROS):
                    pending[m + 1] = emit_load(m + 1)
                emit_groups(m, *pending.pop(m))
    nc.compile()
    return nc


def _prep_weights(filter_banks, hw):
    fb = np.asarray(filter_banks, dtype=np.float32)
    n_mels, n_bins = fb.shape  # (20, 257)
    assert n_mels == N_MELS and n_bins == FRAME // 2 + 1
    assert np.all(fb[:, 0] == 0.0), "parity-fold kernel needs an unused DC bin"

    k_even = np.arange(2, 257, 2)  # 128 bins: 2..256
    k_odd = np.arange(1, 256, 2)  # 128 bins: 1..255
    n256 = np.arange(256, dtype=np.float64)
    n512 = np.arange(512, dtype=np.float64)
    hw64 = np.asarray(hw, dtype=np.float64)
    ce = (hw64[:256, None]
          * np.cos(2.0 * np.pi * np.outer(n256, k_even) / FRAME) / FRAME)
    wo = (hw64[:, None]
          * np.cos(2.0 * np.pi * np.outer(n512, k_odd) / FRAME) / FRAME)
    ce = ce.reshape(2, 128, 128).astype(np.float16)
    wo = wo.reshape(NQ, 128, 128)[CHUNK_ORDER]
    wo = np.ascontiguousarray(wo).astype(np.float16)

    # 2^-75 bias: the on-device eps clamp comes from subnormal flush of
    # mels * 2^-75 (see build_graph docstring). Exact power-of-2 scale.
    fbt = np.empty((2, 128, N_MELS), dtype=ml_dtypes.bfloat16)
    fbt[0] = (fb[:, k_even] * np.float32(2.0**-75)).T
    fbt[1] = (fb[:, k_odd] * np.float32(2.0**-75)).T

    g = (hw64[256:] / hw64[:256]).astype(np.float32)  # [256]
    gr = np.ascontiguousarray(g.reshape(2, 128).T)  # [128, 2]
    return ce, wo, fbt, gr


def _prep_inputs(x):
    """Shard, permute, transpose, cast: per core xt[c, p, r] with the macro-
    local row order r = 128*s + p_out chosen so stores are contiguous."""
    x16 = x.reshape(N_CORES, R, FRAME).astype(np.float16)
    parts = []
    for r0, RB in MACROS:
        S = RB // 128
        blk = x16[:, r0 : r0 + RB, :].reshape(N_CORES, 128, S, FRAME)
        # [core, p, s, n] -> [core, n, s, p] -> [core, NQ, 128, S*128]
        t = blk.transpose(0, 3, 2, 1).reshape(N_CORES, NQ, 128, RB)
        parts.append(t[:, CHUNK_ORDER])
    xt = np.concatenate(parts, axis=3)  # [core, NQ, 128, R]
    return np.ascontiguousarray(xt)


_CACHE = {}


def _get_graph():
    if "nc" not in _CACHE:
        _CACHE["nc"] = build_graph()
    return _CACHE["nc"]


def kernel(inputs, filter_banks, hw, _trace=False):
    x = np.ascontiguousarray(np.asarray(inputs, dtype=np.float32))
    assert x.shape == (B, T, FRAME), x.shape
    ce, wo, fbt, gr = _prep_weights(filter_banks, hw)
    xt = _prep_inputs(x)

    nc = _get_graph()
    in_maps = [
        {"xt": xt[i], "ce": ce, "wo": wo, "fbt": fbt, "gr": gr}
        for i in range(N_CORES)
    ]
    res = run_bass_kernel_spmd(
        nc, in_maps, core_ids=list(range(N_CORES)), trace=_trace
    )
    out = np.stack(
        [np.asarray(res.results[i]["out"]) for i in range(N_CORES)], axis=0
    )
    # bf16 -> f32 is exact for these small-integer outputs
    out = out.astype(np.float32).reshape(B, T, N_MELS, 1)
    if _trace:
        kernel._last_result = res
    return out


# revision 38
# speedup vs baseline: 1.1738x; 1.1738x over previous
"""Trainium2 Bass kernel for AudioPreprocessingLayer.

Computes: floor(log2(mel_fb @ (rfft(x*hamming, norm=forward).real ** 2)))
for x of shape (4096, 32, 512), sharded batch-wise across 8 NeuronCores.

Key ideas:
  - rfft(.).real is a matmul with the cosine matrix C[n,k] = cos(2*pi*k*n/512)/512.
  - Parity fold: C[n+256, k] = (-1)^k C[n, k], so the even-k bins need only
    ue[n] = hw[n]x[n] + hw[n+256]x[n+256] and the odd-k bins only
    uo[n] = hw[n]x[n] - hw[n+256]x[n+256] — a 256-long contraction instead
    of 512: the DFT matmul work halves.
  - Window-in-weights: ue = hw_lo * (x_lo + g*x_hi) with g = hw_hi/hw_lo,
    and the outer hw_lo folds into the cosine weights. The even side is
    folded on the DVE (one 4x-mode multiply + one 2x-mode add per chunk);
    the odd side runs UNFOLDED on the PE straight from the input tiles —
    a full fold would make the DVE the bottleneck.
  - The host hands the kernel x already TRANSPOSED to [n, r] layout (a pure
    permutation, done during sharding), so no on-chip transpose is needed:
    the DMA-loaded tiles feed the DFT matmul directly with n on partitions.
  - The row order within each DMA macro-block is permuted host-side so the
    OUTPUT rows land partition-contiguous (big store descriptors).
  - fp16 end-to-end for x/u and the windowed cosine weights (better
    precision than a bf16 pipeline and full PE speed); mag/filterbank in
    bf16 (fp16 would flush y^2 subnormals); PSUM accumulation in f32.
  - floor(log2(m)): the filterbank is pre-scaled by 2^-75, so f32 subnormal
    flush implements the eps clamp and the result is just
    (bitcast_int32(mels) >> 23) - 52, stored as bf16 (exact small ints).
"""

import os
import sys

for _p in ("/opt/trn_rl_repo",):
    if _p not in sys.path and os.path.isdir(_p):
        sys.path.append(_p)

import numpy as np
import ml_dtypes

import concourse.bass as bass
from concourse import bacc, mybir
from concourse.tile import TileContext
from concourse.bass_utils import run_bass_kernel_spmd

N_CORES = 8
B, T, FRAME = 4096, 32, 512
R = (B // N_CORES) * T  # 16384 rows of length 512 per core
N_MELS = 20
NQ = FRAME // 128  # 4 n-chunks of the transposed input
GR = 512  # rows per compute group (one PSUM bank of f32)
CHUNK_ORDER = [0, 2, 1, 3]  # n-chunk storage order: fold pairs adjacent

# DMA macro-blocks (rows): small first blocks so the pipeline fills quickly,
# and a smaller last block so the drain tail is short.
MACROS = [(0, 256), (256, 768), (1024, 1024)] + [
    (2048 + 2048 * i, 2048) for i in range(6)
] + [(14336, 1024), (15360, 1024)]
assert sum(rb for _, rb in MACROS) == R

f32 = mybir.dt.float32
f16 = mybir.dt.float16
bf16 = mybir.dt.bfloat16
i32 = mybir.dt.int32


def build_graph():
    """SPMD Bass graph for one core's shard.

    xt:  [NQ, 128, R] f16   transposed rows, n-chunks stored in order
         [0,2,1,3] so each half [0,2] / [1,3] is one contiguous DMA that
         feeds one fold: xt[i, p, r] = x[perm(r), 128*chunk(i)+p]
    ce:  [2, 128, 128] f16  diag(hw_lo) @ cos matrix, even k (2,4,...,256)
    wo:  [NQ, 128, 128] f16 full windowed cos matrix, odd k (1,3,...,255),
         n-chunks in the same [0,2,1,3] order as xt
    fbt: [2, 128, N_MELS] bf16  mel filterbank * 2^-75, split by k parity
         (the 2^-75 bias makes f32 subnormal flush implement the eps clamp:
          floor(log2(mels)) = (expbits(mels * 2^-75) >> 23) - 52, exact for
          mels > 2^-51, and the mels==0 -> eps path lands on -52 via the
          zero/subnormal exponent field)
    gr:  [128, 2] f32       window ratio hw_hi/hw_lo, n-chunked
    out: [R, N_MELS] bf16   (exact small ints; host converts to f32)
    """
    nc = bacc.Bacc(None, target_bir_lowering=False)
    xt_d = nc.declare_dram_parameter("xt", [NQ, 128, R], f16, isOutput=False)
    ce_d = nc.declare_dram_parameter("ce", [2, 128, 128], f16, isOutput=False)
    wo_d = nc.declare_dram_parameter("wo", [NQ, 128, 128], f16, isOutput=False)
    fbt_d = nc.declare_dram_parameter("fbt", [2, 128, N_MELS], bf16, isOutput=False)
    g_d = nc.declare_dram_parameter("gr", [128, 2], f32, isOutput=False)
    out_d = nc.declare_dram_parameter("out", [R, N_MELS], bf16, isOutput=True)

    with TileContext(nc) as tc:
        with (
            tc.tile_pool(name="consts", bufs=1) as consts,
            tc.tile_pool(name="xta", bufs=4) as xta_pool,
            tc.tile_pool(name="xtb", bufs=4) as xtb_pool,
            tc.tile_pool(name="gx", bufs=3) as gx_pool,
            tc.tile_pool(name="u", bufs=3) as u_pool,
            tc.tile_pool(name="mag", bufs=3) as mag_pool,
            tc.tile_pool(name="fin", bufs=2) as fin_pool,
            tc.tile_pool(name="ps_y", bufs=3, space="PSUM") as ps_y_pool,
            tc.tile_pool(name="ps_m", bufs=2, space="PSUM") as ps_m_pool,
        ):
            # gr first (unblocks the folds), then matmul weights; macro
            # input DMAs follow right behind these on the sync queue.
            g_sb = consts.tile([128, 2], f32)
            nc.sync.dma_start(out=g_sb, in_=g_d[:, :])
            wo_sb = consts.tile([128, NQ, 128], f16)
            nc.sync.dma_start(out=wo_sb, in_=wo_d.rearrange("c p k -> p c k"))
            ce_sb = consts.tile([128, 2, 128], f16)
            nc.sync.dma_start(out=ce_sb, in_=ce_d.rearrange("c p k -> p c k"))
            fbt_sb = consts.tile([128, 2, N_MELS], bf16)
            nc.sync.dma_start(out=fbt_sb, in_=fbt_d.rearrange("e j m -> j e m"))

            def emit_load(m):
                """Two half-DMAs + even-side window/fold for macro m. Each
                half carries the (x_lo, x_hi) pair one fold needs, so the
                folds start after half the macro's data has landed."""
                r0, RB = MACROS[m]
                xta_sb = xta_pool.tile([128, 2, RB], f16, name="xta_sb")
                nc.gpsimd.dma_start(
                    out=xta_sb,
                    in_=xt_d[0:2, :, r0 : r0 + RB].rearrange("c p r -> p c r"),
                )
                xtb_sb = xtb_pool.tile([128, 2, RB], f16, name="xtb_sb")
                nc.sync.dma_start(
                    out=xtb_sb,
                    in_=xt_d[2:4, :, r0 : r0 + RB].rearrange("c p r -> p c r"),
                )
                # u[c] = x[c] + g[c]*x[c+2]  (hw_lo is folded into ce;
                # the odd side runs unfolded on the PE straight from xt)
                gx_sb = gx_pool.tile([128, 2, RB], f16, name="gx_sb")
                u_sb = u_pool.tile([128, 2, RB], f16, name="u_sb")
                for c, h_sb in ((0, xta_sb), (1, xtb_sb)):
                    nc.vector.tensor_scalar(
                        gx_sb[:, c], h_sb[:, 1], g_sb[:, c : c + 1],
                        None, mybir.AluOpType.mult,
                    )
                    nc.vector.tensor_add(u_sb[:, c], h_sb[:, 0], gx_sb[:, c])
                return (xta_sb, xtb_sb), u_sb

            def emit_groups(m, xt_sb, u_sb):
                xta_sb, xtb_sb = xt_sb
                r0, RB = MACROS[m]
                S = RB // 128  # output slots per macro
                mels_ps = ps_m_pool.tile([128, S * N_MELS], f32, name="mels_ps")
                for off in range(0, RB, GR):
                    gr_n = min(GR, RB - off)
                    r = slice(off, off + gr_n)
                    # DFT: y[k, r] for even/odd k (f32 PSUM accumulate);
                    # odd first — it reads xt directly, no DVE dependency.
                    y_ps = ps_y_pool.tile([128, 2, gr_n], f32, name="y_ps")
                    odd_srcs = (xta_sb[:, 0, r], xta_sb[:, 1, r],
                                xtb_sb[:, 0, r], xtb_sb[:, 1, r])
                    for c in range(NQ):
                        nc.tensor.matmul(
                            y_ps[:, 1, :], wo_sb[:, c, :], odd_srcs[c],
                            start=(c == 0), stop=(c == NQ - 1),
                        )
                    for c in range(2):
                        nc.tensor.matmul(
                            y_ps[:, 0, :], ce_sb[:, c, :], u_sb[:, c, r],
                            start=(c == 0), stop=(c == 1),
                        )
                    # mag = y^2 (fused PSUM -> SBUF bf16)
                    mag_sb = mag_pool.tile([128, 2, gr_n], bf16, name="mag_sb")
                    nc.scalar.activation(
                        mag_sb, y_ps, mybir.ActivationFunctionType.Square
                    )
                    # mel: mels[r, m] += mag[k, r].T @ fbt[k, m]
                    # (a whole macro's mels fit one PSUM bank)
                    for j in range(gr_n // 128):
                        jj = slice(j * 128, (j + 1) * 128)
                        s = off // 128 + j
                        for e in range(2):
                            nc.tensor.matmul(
                                mels_ps[:, s * N_MELS : (s + 1) * N_MELS],
                                mag_sb[:, e, jj], fbt_sb[:, e, :],
                                start=(e == 0), stop=(e == 1),
                            )
                # finalize: floor(log2(mels)) = expbits(mels * 2^-75) - 52
                e_sb = fin_pool.tile([128, S * N_MELS], i32, tag="e_sb",
                                     name="e_sb")
                nc.vector.tensor_scalar(
                    e_sb,
                    mels_ps.bitcast(i32),
                    23,
                    None,
                    mybir.AluOpType.logical_shift_right,
                )
                o_sb = fin_pool.tile([128, S * N_MELS], bf16, tag="o_sb",
                                     name="o_sb")
                nc.vector.tensor_scalar_sub(o_sb, e_sb, 52.0)
                # store: rows r0 + p*S + s are partition-contiguous in DRAM
                q = nc.sync if m % 2 == 0 else nc.gpsimd
                q.dma_start(
                    out=out_d[r0 : r0 + RB, :].rearrange(
                        "(p j) m -> p (j m)", j=S
                    ),
                    in_=o_sb,
                )

            # software pipeline: load macro m+1 (DMA + DVE folds) before
            # emitting macro m's matmul groups, so the DVE FIFO never parks
            # next macro's folds behind this macro's exponent shifts.
            pending = {0: emit_load(0)}
            for m in range(len(MACROS)):
                if m + 1 < len(MACROS):
                    pending[m + 1] = emit_load(m + 1)
                emit_groups(m, *pending.pop(m))
    nc.compile()
    return nc


def _prep_weights(filter_banks, hw):
    fb = np.asarray(filter_banks, dtype=np.float32)
    n_mels, n_bins = fb.shape  # (20, 257)
    assert n_mels == N_MELS and n_bins == FRAME // 2 + 1
    assert np.all(fb[:, 0] == 0.0), "parity-fold kernel needs an unused DC bin"

    k_even = np.arange(2, 257, 2)  # 128 bins: 2..256
    k_odd = np.arange(1, 256, 2)  # 128 bins: 1..255
    n256 = np.arange(256, dtype=np.float64)
    n512 = np.arange(512, dtype=np.float64)
    hw64 = np.asarray(hw, dtype=np.float64)
    ce = (hw64[:256, None]
          * np.cos(2.0 * np.pi * np.outer(n256, k_even) / FRAME) / FRAME)
    wo = (hw64[:, None]
          * np.cos(2.0 * np.pi * np.outer(n512, k_odd) / FRAME) / FRAME)
    ce = ce.reshape(2, 128, 128).astype(np.float16)
    wo = wo.reshape(NQ, 128, 128)[CHUNK_ORDER]
    wo = np.ascontiguousarray(wo).astype(np.float16)

    # 2^-75 bias: the on-device eps clamp comes from subnormal flush of
    # mels * 2^-75 (see build_graph docstring). Exact power-of-2 scale.
    fbt = np.empty((2, 128, N_MELS), dtype=ml_dtypes.bfloat16)
    fbt[0] = (fb[:, k_even] * np.float32(2.0**-75)).T
    fbt[1] = (fb[:, k_odd] * np.float32(2.0**-75)).T

    g = (hw64[256:] / hw64[:256]).astype(np.float32)  # [256]
    gr = np.ascontiguousarray(g.reshape(2, 128).T)  # [128, 2]
    return ce, wo, fbt, gr


def _prep_inputs(x):
    """Shard, permute, transpose, cast: per core xt[c, p, r] with the macro-
    local row order r = 128*s + p_out chosen so stores are contiguous."""
    x16 = x.reshape(N_CORES, R, FRAME).astype(np.float16)
    parts = []
    for r0, RB in MACROS:
        S = RB // 128
        blk = x16[:, r0 : r0 + RB, :].reshape(N_CORES, 128, S, FRAME)
        # [core, p, s, n] -> [core, n, s, p] -> [core, NQ, 128, S*128]
        t = blk.transpose(0, 3, 2, 1).reshape(N_CORES, NQ, 128, RB)
        parts.append(t[:, CHUNK_ORDER])
    xt = np.concatenate(parts, axis=3)  # [core, NQ, 128, R]
    return np.ascontiguousarray(xt)


_CACHE = {}


def _get_graph():
    if "nc" not in _CACHE:
        _CACHE["nc"] = build_graph()
    return _CACHE["nc"]


def kernel(inputs, filter_banks, hw, _trace=False):
    x = np.ascontiguousarray(np.asarray(inputs, dtype=np.float32))
    assert x.shape == (B, T, FRAME), x.shape
    ce, wo, fbt, gr = _prep_weights(filter_banks, hw)
    xt = _prep_inputs(x)

    nc = _get_graph()
    in_maps = [
        {"xt": xt[i], "ce": ce, "wo": wo, "fbt": fbt, "gr": gr}
        for i in range(N_CORES)
    ]
    res = run_bass_kernel_spmd(
        nc, in_maps, core_ids=list(range(N_CORES)), trace=_trace
    )
    out = np.stack(
        [np.asarray(res.results[i]["out"]) for i in range(N_CORES)], axis=0
    )
    # bf16 -> f32 is exact for these small-integer outputs
    out = out.astype(np.float32).reshape(B, T, N_MELS, 1)
    if _trace:
        kernel._last_result = res
    return out
